# revision 9
# baseline (speedup 1.0000x reference)
"""Bias multi-head attention (ALiBi + additive bias + causal) on 8 Trainium2
NeuronCores.

Sharding: data parallel over batch (B=2) x tensor parallel over heads
(16 heads -> 4 per core). Each core computes QKV projections for its 4 heads,
causal attention with the additive bias, and a partial output projection;
the host sums the 4 partials per batch and adds the output bias.

Math notes (exact reductions of the reference):
 - ALiBi term -slope*max(j-i,0) is nonzero only where j>i, which the causal
   mask sets to -inf, so ALiBi vanishes entirely.
 - k-bias bk shifts every logit of a row by q_m . bk (constant in j), which
   softmax is invariant to -> dropped.
 - v-bias bv contributes bv @ Wo_slice.T after normalization -> added on host.
 - Softmax is computed without max-subtraction (logits are O(10), exp is safe
   in fp32); the denominator comes from a ones-column appended to V.
 - attn_bias enters as a precomputed exp(bias^T) multiplier after exp(S/8),
   with causal zeros baked into the diagonal blocks.

Device dataflow per core (P=128 blocks, N=2048, D=1024, hd=64, 4 heads):
 - qT/kT [dlocal, m] and v [j, dlocal] from bf16 matmuls vs pre-transposed
   host inputs (xT, W.T slices).
 - S^T[j, m] = kT_tile.T @ qT (contraction over d=64; two heads packed on
   PE row groups 0-63 / 64-127).
 - P^T = exp(S^T/8) * expbias^T  (ACT exp + DVE mul, bf16).
 - O[m, 65] += P^T_tile.T @ [v_h | 1]  (denominator in column 64).
 - normalize, transpose O via PE, partial out = O^T.T @ Wo_slice^T.
"""

import math
import os
import sys

for _p in ("/opt/trn_rl_repo",):
    if _p not in sys.path:
        sys.path.insert(0, _p)

import numpy as np
import ml_dtypes

B, N, D = 2, 2048, 1024
H, HD = 16, 64
P = 128
NB = N // P              # 16 m/j blocks
HPC = 4                  # heads per core
DC = HPC * HD            # 256 local head dims
NCORES = 8
GJ = 8                   # j-tiles per softmax strip (2 PSUM banks)

bf16 = ml_dtypes.bfloat16

_CACHE = {}


def _build_nc(dbg=False):
    import concourse.bacc as bacc
    import concourse.mybir as mybir
    import concourse.tile as tile
    from concourse.masks import make_identity

    f32 = mybir.dt.float32
    bf = mybir.dt.bfloat16
    Copy = mybir.ActivationFunctionType.Copy
    Exp = mybir.ActivationFunctionType.Exp

    nc = bacc.Bacc("TRN2", target_bir_lowering=False, debug=False)
    if dbg:
        qT_dump = nc.dram_tensor("qT_dump", [P, 2, N], mybir.dt.bfloat16, kind="ExternalOutput")
        kT_dump = nc.dram_tensor("kT_dump", [P, 2, N], mybir.dt.bfloat16, kind="ExternalOutput")
        v_dump = nc.dram_tensor("v_dump", [P, NB, HPC, HD + 1], mybir.dt.bfloat16, kind="ExternalOutput")
        on_dump = nc.dram_tensor("on_dump", [NB, P, HPC, HD], mybir.dt.bfloat16, kind="ExternalOutput")
        ot_dump = nc.dram_tensor("ot_dump", [NB, P, 2, P], mybir.dt.bfloat16, kind="ExternalOutput")

    xqT_d = nc.dram_tensor("xqT", [D, N], bf, kind="ExternalInput")
    xkvT_d = nc.dram_tensor("xkvT", [D, N], bf, kind="ExternalInput")
    wqT_d = nc.dram_tensor("wqT", [D, DC], bf, kind="ExternalInput")
    wkT_d = nc.dram_tensor("wkT", [D, DC], bf, kind="ExternalInput")
    wvT_d = nc.dram_tensor("wvT", [D, DC], bf, kind="ExternalInput")
    woT_d = nc.dram_tensor("woT", [DC, D], bf, kind="ExternalInput")
    bq_d = nc.dram_tensor("bq", [DC], f32, kind="ExternalInput")
    ebT_d = nc.dram_tensor("ebT", [N, N], bf, kind="ExternalInput")
    outp_d = nc.dram_tensor("outp", [N, D], f32, kind="ExternalOutput")

    ET = D // P  # 8 contraction tiles over the model dim

    with tile.TileContext(nc) as tc:
        with (
            tc.tile_pool(name="const", bufs=1) as const,
            tc.tile_pool(name="xp", bufs=10) as xp,
            tc.tile_pool(name="ebp", bufs=4) as ebp,
            tc.tile_pool(name="pp", bufs=4) as pp,
            tc.tile_pool(name="onp", bufs=2) as onp,
            tc.tile_pool(name="otp", bufs=3) as otp,
            tc.tile_pool(name="rp", bufs=8) as rp,
            tc.tile_pool(name="outs", bufs=2) as outs,
            tc.tile_pool(name="spp", bufs=2, space="PSUM") as spp,
            tc.tile_pool(name="opp", bufs=2, space="PSUM") as opp,
            tc.tile_pool(name="cpp", bufs=2, space="PSUM") as cpp,
        ):
            # ---- constants -------------------------------------------------
            wq_sb = const.tile([P, ET, DC], bf, name="wq_sb")
            wk_sb = const.tile([P, ET, DC], bf, name="wk_sb")
            wv_sb = const.tile([P, ET, DC], bf, name="wv_sb")
            nc.sync.dma_start(out=wq_sb, in_=wqT_d[:, :].rearrange("(et p) d -> p et d", p=P))
            nc.sync.dma_start(out=wk_sb, in_=wkT_d[:, :].rearrange("(et p) d -> p et d", p=P))
            nc.sync.dma_start(out=wv_sb, in_=wvT_d[:, :].rearrange("(et p) d -> p et d", p=P))
            wo_sb = const.tile([P, 2, D], bf, name="wo_sb")
            nc.sync.dma_start(out=wo_sb, in_=woT_d[:, :].rearrange("(c p) e -> p c e", p=P))
            bq_sb = const.tile([P, 2], f32, name="bq_sb")
            nc.sync.dma_start(out=bq_sb, in_=bq_d[:].rearrange("(c p) -> p c", p=P))
            idy = const.tile([P, P], bf, name="idy")
            make_identity(nc, idy)

            qT = const.tile([P, 2, N], bf, name="qT")    # [2 heads/chunk, m]
            kT = const.tile([P, 2, N], bf, name="kT")
            v = const.tile([P, NB, HPC, HD + 1], bf, name="v")  # [j, jt, h, d|1]
            nc.vector.memset(v[:, :, :, HD:HD + 1], 1.0)

            # ---- Phase A: projections -------------------------------------
            for mg in range(4):
                msl = slice(mg * 512, (mg + 1) * 512)
                xq_t = []
                for et in range(ET):
                    xt = xp.tile([P, 512], bf, name="xq_t", tag="xt")
                    nc.sync.dma_start(out=xt, in_=xqT_d[et * P:(et + 1) * P, msl])
                    xq_t.append(xt)
                for c in range(2):
                    ps = spp.tile([P, GJ, P], f32, name="ps_q", tag="sp")
                    for et in range(ET):
                        nc.tensor.matmul(
                            ps[:, 0:4, :].rearrange("p a b -> p (a b)"),
                            wq_sb[:, et, c * P:(c + 1) * P],
                            xq_t[et],
                            start=(et == 0), stop=(et == ET - 1),
                        )
                    nc.vector.tensor_scalar_add(
                        qT[:, c, msl],
                        ps[:, 0:4, :].rearrange("p a b -> p (a b)"),
                        bq_sb[:, c:c + 1],
                    )
            for mg in range(4):
                msl = slice(mg * 512, (mg + 1) * 512)
                xkv_t = []
                for et in range(ET):
                    xt = xp.tile([P, 512], bf, name="xkv_t", tag="xt")
                    nc.sync.dma_start(out=xt, in_=xkvT_d[et * P:(et + 1) * P, msl])
                    xkv_t.append(xt)
                for c in range(2):
                    ps = spp.tile([P, GJ, P], f32, name="ps_k", tag="sp")
                    for et in range(ET):
                        nc.tensor.matmul(
                            ps[:, 0:4, :].rearrange("p a b -> p (a b)"),
                            wk_sb[:, et, c * P:(c + 1) * P],
                            xkv_t[et],
                            start=(et == 0), stop=(et == ET - 1),
                        )
                    nc.any.tensor_copy(
                        kT[:, c, msl], ps[:, 0:4, :].rearrange("p a b -> p (a b)")
                    )
                for jl in range(4):
                    jt = mg * 4 + jl
                    psv = cpp.tile([P, 512], f32, name="ps_v", tag="cp")
                    for et in range(ET):
                        nc.tensor.matmul(
                            psv[:, 0:DC],
                            xkv_t[et][:, jl * P:(jl + 1) * P],
                            wv_sb[:, et, :],
                            start=(et == 0), stop=(et == ET - 1),
                        )
                    nc.any.tensor_copy(
                        v[:, jt, :, 0:HD],
                        psv[:, 0:DC].rearrange("p (h d) -> p h d", h=HPC),
                    )

            if dbg:
                nc.sync.dma_start(out=qT_dump[:, :, :], in_=qT)
                nc.sync.dma_start(out=kT_dump[:, :, :], in_=kT)
                nc.sync.dma_start(out=v_dump[:, :, :, :], in_=v)

            # ---- Phase B: attention ---------------------------------------
            mul_rr = 0
            for mt in range(NB):
                msl = slice(mt * P, (mt + 1) * P)
                n_j = mt + 1
                # expbias tiles for this m block, shared by both head pairs
                ebts = []
                for s0 in range(0, n_j, GJ):
                    g = min(GJ, n_j - s0)
                    ebt = ebp.tile([P, GJ, P], bf, name="ebt", tag="eb")
                    nc.sync.dma_start(
                        out=ebt[:, 0:g, :],
                        in_=ebT_d[s0 * P:(s0 + g) * P, msl].rearrange(
                            "(g p) m -> p g m", p=P),
                    )
                    ebts.append(ebt)
                on = onp.tile([P, HPC, HD], bf, name="on")
                for hp in range(2):
                    hA, hB = 2 * hp, 2 * hp + 1
                    # one PSUM bank per head: an accumulation group's `start`
                    # marks its whole 2KB zero-region, so heads cannot share
                    oA = opp.tile([P, P], f32, name="oA", tag="op")
                    oB = opp.tile([P, P], f32, name="oB", tag="op")
                    for si, s0 in enumerate(range(0, n_j, GJ)):
                        g = min(GJ, n_j - s0)
                        ebt = ebts[si]
                        sA = spp.tile([P, GJ, P], f32, name="sA", tag="sp")
                        sB = spp.tile([P, GJ, P], f32, name="sB", tag="sp")
                        for ji in range(g):
                            jsl = slice((s0 + ji) * P, (s0 + ji + 1) * P)
                            nc.tensor.matmul(
                                sA[:, ji, :], kT[0:64, hp, jsl], qT[0:64, hp, msl],
                                start=True, stop=True)
                            nc.tensor.matmul(
                                sB[:, ji, :], kT[64:128, hp, jsl], qT[64:128, hp, msl],
                                start=True, stop=True)
                        pA = pp.tile([P, GJ, P], bf, name="pA", tag="pt")
                        pB = pp.tile([P, GJ, P], bf, name="pB", tag="pt")
                        nc.scalar.activation(
                            pA[:, 0:g, :].rearrange("p a b -> p (a b)"),
                            sA[:, 0:g, :].rearrange("p a b -> p (a b)"),
                            Exp, scale=1.0 / math.sqrt(HD))
                        nc.scalar.activation(
                            pB[:, 0:g, :].rearrange("p a b -> p (a b)"),
                            sB[:, 0:g, :].rearrange("p a b -> p (a b)"),
                            Exp, scale=1.0 / math.sqrt(HD))
                        ebf = ebt[:, 0:g, :].rearrange("p a b -> p (a b)")
                        for p_t in (pA, pB):
                            pf = p_t[:, 0:g, :].rearrange("p a b -> p (a b)")
                            if mul_rr % 3 == 2:
                                nc.gpsimd.tensor_mul(pf, pf, ebf)
                            else:
                                nc.vector.tensor_mul(pf, pf, ebf)
                            mul_rr += 1
                        for ji in range(g):
                            jt = s0 + ji
                            nc.tensor.matmul(
                                oA[:, 0:HD + 1], pA[:, ji, :], v[:, jt, hA, :],
                                start=(jt == 0), stop=(jt == n_j - 1))
                            nc.tensor.matmul(
                                oB[:, 0:HD + 1], pB[:, ji, :], v[:, jt, hB, :],
                                start=(jt == 0), stop=(jt == n_j - 1))
                    # normalize this head pair
                    for h, o_ps in ((hA, oA), (hB, oB)):
                        r = rp.tile([P, 1], f32, name="r")
                        nc.vector.reciprocal(r, o_ps[:, HD:HD + 1])
                        nc.vector.tensor_scalar_mul(on[:, h, :], o_ps[:, 0:HD], r)
                # transpose O -> OT [dc, m]
                ot = otp.tile([P, 2, P], bf, name="ot")
                onf = on.rearrange("p h d -> p (h d)")
                for c in range(2):
                    t_ps = cpp.tile([P, P], bf, name="t_ps", tag="cp")
                    nc.tensor.transpose(t_ps, onf[:, c * P:(c + 1) * P], idy)
                    nc.any.tensor_copy(ot[:, c, :], t_ps)
                if dbg:
                    nc.sync.dma_start(out=on_dump[mt, :, :, :], in_=on)
                    nc.sync.dma_start(out=ot_dump[mt, :, :, :], in_=ot)
                # output projection for this m block
                osb = outs.tile([P, 2, 512], f32, name="osb")
                for eg in range(2):
                    c_ps = cpp.tile([P, 512], f32, name="c_ps", tag="cp")
                    for c in range(2):
                        nc.tensor.matmul(
                            c_ps, ot[:, c, :], wo_sb[:, c, eg * 512:(eg + 1) * 512],
                            start=(c == 0), stop=(c == 1))
                    nc.any.tensor_copy(osb[:, eg, :], c_ps)
                nc.sync.dma_start(
                    out=outp_d[msl, :], in_=osb.rearrange("p a b -> p (a b)"))

    nc.compile()
    return nc


def _get_nc():
    if "nc" not in _CACHE:
        _CACHE["nc"] = _build_nc()
    return _CACHE["nc"]


def _host_prep(x_q, x_kv, attn_bias, Wq, bq, Wk, Wv, Wo):
    """Build the 8 per-core input maps."""
    xqT = [np.ascontiguousarray(x_q[b].T).astype(bf16) for b in range(B)]
    xkvT = [np.ascontiguousarray(x_kv[b].T).astype(bf16) for b in range(B)]
    ebT = np.exp(attn_bias.astype(np.float32)).T
    # causal zeros inside diagonal blocks: ebT[j, m] masked where j > m
    ztri = np.tril(np.ones((P, P), dtype=bool), k=-1)
    ebT = np.ascontiguousarray(ebT)
    for t in range(NB):
        blk = ebT[t * P:(t + 1) * P, t * P:(t + 1) * P]
        blk[ztri] = 0.0
    ebT = ebT.astype(bf16)

    in_maps = []
    for core in range(NCORES):
        b = core // 4
        hg = core % 4
        hsl = slice(hg * DC, (hg + 1) * DC)
        in_maps.append({
            "xqT": xqT[b],
            "xkvT": xkvT[b],
            "wqT": np.ascontiguousarray(Wq[hsl, :].T).astype(bf16),
            "wkT": np.ascontiguousarray(Wk[hsl, :].T).astype(bf16),
            "wvT": np.ascontiguousarray(Wv[hsl, :].T).astype(bf16),
            "woT": np.ascontiguousarray(Wo[:, hsl].T).astype(bf16),
            "bq": np.ascontiguousarray(bq[hsl]).astype(np.float32),
            "ebT": ebT,
        })
    return in_maps


def _run(inputs, trace=False):
    """Run the SPMD kernel; returns (out [B,N,D] fp32, BassKernelResults)."""
    from concourse.bass_utils import run_bass_kernel_spmd

    x_q = np.asarray(inputs["x_q"], dtype=np.float32)
    x_kv = np.asarray(inputs["x_kv"], dtype=np.float32)
    attn_bias = np.asarray(inputs["attn_bias"], dtype=np.float32)
    Wq = np.asarray(inputs["Wq"], dtype=np.float32)
    bq = np.asarray(inputs["bq"], dtype=np.float32)
    Wk = np.asarray(inputs["Wk"], dtype=np.float32)
    Wv = np.asarray(inputs["Wv"], dtype=np.float32)
    bv = np.asarray(inputs["bv"], dtype=np.float32)
    Wo = np.asarray(inputs["Wo"], dtype=np.float32)
    bo = np.asarray(inputs["bo"], dtype=np.float32)

    nc = _get_nc()
    in_maps = _host_prep(x_q, x_kv, attn_bias, Wq, bq, Wk, Wv, Wo)
    res = run_bass_kernel_spmd(nc, in_maps, core_ids=list(range(NCORES)),
                               trace=trace)
    out = np.zeros((B, N, D), dtype=np.float32)
    for core in range(NCORES):
        out[core // 4] += res.results[core]["outp"]
    out += (bo + bv @ Wo.T)[None, None, :]
    return out, res


def _reference_numpy(x_q, x_kv, attn_bias, Wq, bq, Wk, bk, Wv, bv, Wo, bo,
                     is_self_attn, causal):
    """Fallback for configurations the device kernel doesn't cover."""
    def slopes(n):
        start = 2.0 ** (-(2.0 ** (-(math.log2(n) - 3))))
        return np.array([start * start ** i for i in range(n)], dtype=np.float32)

    Bq, Nq, _ = x_q.shape
    Nk = x_kv.shape[1]
    q = (x_q @ Wq.T + bq).reshape(Bq, Nq, H, HD)
    k = (x_kv @ Wk.T + bk).reshape(Bq, Nk, H, HD)
    vv = (x_kv @ Wv.T + bv).reshape(Bq, Nk, H, HD)
    logits = np.einsum("bqhd,bkhd->bhqk", q, k) / math.sqrt(HD)
    if is_self_attn and Nq == Nk:
        dist = np.maximum(np.arange(Nk)[None, :] - np.arange(Nq)[:, None], 0)
        logits = logits - slopes(H)[None, :, None, None] * dist[None, None]
    if attn_bias is not None:
        logits = logits + attn_bias[None, None]
    if causal and is_self_attn and Nq == Nk:
        mask = np.triu(np.ones((Nq, Nk), dtype=bool), k=1)
        logits = np.where(mask[None, None], -np.inf, logits)
    logits -= logits.max(axis=-1, keepdims=True)
    e = np.exp(logits)
    attn = e / e.sum(axis=-1, keepdims=True)
    out = np.einsum("bhqk,bkhd->bqhd", attn, vv).reshape(Bq, Nq, -1)
    return out @ Wo.T + bo


def kernel(**inputs):
    is_self = int(np.asarray(inputs.get("is_self_attn", 1)))
    causal = int(np.asarray(inputs.get("causal", 1)))
    if not (is_self and causal):
        return _reference_numpy(
            np.asarray(inputs["x_q"], np.float32),
            np.asarray(inputs["x_kv"], np.float32),
            np.asarray(inputs["attn_bias"], np.float32),
            np.asarray(inputs["Wq"], np.float32), np.asarray(inputs["bq"], np.float32),
            np.asarray(inputs["Wk"], np.float32), np.asarray(inputs["bk"], np.float32),
            np.asarray(inputs["Wv"], np.float32), np.asarray(inputs["bv"], np.float32),
            np.asarray(inputs["Wo"], np.float32), np.asarray(inputs["bo"], np.float32),
            is_self, causal).astype(np.float32)
    out, _ = _run(inputs, trace=False)
    return out
